# revision 29
# baseline (speedup 1.0000x reference)
"""Multi-head attention kernel for Trainium2, data-parallel over batch on 8 cores.

Problem: B=16, N=1024, DIM=768, H=12 heads, head_dim=64, fp32.
  q = x@Wq+bq; k = x@Wk+bk; v = x@Wv+bv   (per-head split)
  out = softmax(q k^T / sqrt(DIM)) v      (per head), concat, @Wo + bo
Sharding: batch-parallel. Each core gets 2 batches and all weights; no
collectives. Output gathered by concat.

All matmuls in bf16 (inputs/weights host-cast to bf16; fp32 psum accumulate).
Measured output error vs the fp32 reference: ~3.8e-3 of the output absmax.

Per-core layout strategy (per batch of 1024 tokens):
  - XT = x^T  [768 feat, 1024 tok] via XBAR DMA transpose directly from the
    bf16 x in DRAM (no PE transposes, no psum).
  - QT/KT = (x@W + b)^T [768, 1024]: matmul(lhsT=W, rhs=XT). Head h lives on
    partition rows (h%2)*64..: pair p = m-tile p.
  - V natural [1024 tok, 768] via matmul(lhsT=XT, rhs=Wv), stored per-pair
    padded: [Vh0(64) | ones(1) | pad(31) | Vh1(64)] = 160 cols. The shared
    ones column makes PV emit softmax denominators at 32-aligned psum rows:
      h0: lhsT cols [0:128]  -> psum rows 0-63 = O_h0^T, row 64 = denom_h0
      h1: lhsT cols [32:160] -> psum row 32 = denom_h1, rows 64-127 = O_h1^T
  - S^T[key, q] = matmul(lhsT=KT head rows, rhs=QT head rows), contraction 64,
    two heads row-packed in the PE array (partitions 0-63 / 64-127, run
    concurrently).
  - P^T = exp(SCALE * S^T) on ACT (no max subtraction needed: |SCALE*S| < ~2),
    [128, 1024] ops (2 key-blocks per op) to amortize ACT overhead.
  - O^T normalized by broadcast reciprocal rows, written to OT [768, 1024].
  - Y = matmul(lhsT=OT, rhs=Wo) + bo -> natural [tok, 768], DMA out fp32.

Scheduling (the perf-critical part; HW-measured 369us vs 505us for the
naive serial structure):
  - The two batches are software-pipelined at emission level so the PE never
    idles long enough to re-throttle the HAM clock gate (idle >3.4us drops
    the PE clock 2.4->1.2 GHz): batch-1's XBAR transposes + V projection are
    interleaved into batch-0's ACT(exp)-limited attention pairs; batch-0's Y
    projection is interleaved into batch-1's attention pairs.
  - The softmax epilogue (denominator DMA-roundtrip reshape -> reciprocal ->
    broadcast -> normalize) runs entirely on the otherwise-idle GPSIMD
    engine (Newton-iteration reciprocal with a bit-trick seed). Keeping any
    latency-bound op off the busy in-order DVE queue matters: a reciprocal
    waiting on the roundtrip at the DVE queue head blocks the psum-drain
    copies behind it, which hold the PV psum banks and starve the PE
    (measured 10-23us stalls).
  - During startup (batch-0 V) and tail (batch-1 Y) the projection psum
    tiles rotate across the idle attention banks (st/oa/ob), removing the
    2-bank serialization against the DVE drain of the previous tile.
  - QK projections are qs-outer so each macro-op holds one psum bank.
DMA queue assignment: sync = XBAR transposes + denominator roundtrips;
scalar = Wv + biases + out; gpsimd = Wq/Wk/Wo.
"""

import sys
import types

sys.path.insert(0, "/opt/trn_rl_repo")

import numpy as np

# Register the axon NTFF profile hook if the image's antenv lacks it (needed
# only when run with trace=True; harmless otherwise).
import antenv  # noqa: F401

if "antenv.axon_hooks" not in sys.modules:
    _hooks_mod = types.ModuleType("antenv.axon_hooks")
    _hooks_mod._hook = None

    def _set_hook(h):
        _hooks_mod._hook = h

    def _get_hook():
        return _hooks_mod._hook

    _hooks_mod.set_axon_ntff_profile_hook = _set_hook
    _hooks_mod.get_axon_ntff_profile_hook = _get_hook
    sys.modules["antenv.axon_hooks"] = _hooks_mod
    try:
        from trn_agent_boot.trn_boot import _ntff_profile_via_ctypes

        _set_hook(_ntff_profile_via_ctypes("/opt/axon/libaxon_pjrt.so"))
    except Exception:
        pass

import concourse.bass_utils as bass_utils

bass_utils.upload_artifacts = lambda tmpdir: f"local:{tmpdir}"  # no bucket creds

import concourse.bacc as bacc
import concourse.mybir as mybir
import concourse.tile as tile
from concourse.bass_utils import run_bass_kernel_spmd

P = 128
DIM = 768
N_HEADS = 12
HD = 64
N = 1024
B = 16
NCORES = 8
BL = B // NCORES  # batches per core = 2
SCALE = 1.0 / float(np.sqrt(DIM))

KT = DIM // P      # 6 k-tiles of the 768 contraction
TT = N // P        # 8 token tiles per batch
NPAIR = N_HEADS // 2  # 6 head pairs
QC = 512           # query chunk (psum bank, fp32)
PAIRW = 160        # pair block in V_ext: [Vh0(64)|ones(1)|pad(31)|Vh1(64)]

F32 = mybir.dt.float32
BF16 = mybir.dt.bfloat16

_cache = {}


def build():
    nc = bacc.Bacc("TRN2", target_bir_lowering=False, debug=False)

    x = nc.dram_tensor("inputs", [BL, N, DIM], BF16, kind="ExternalInput")
    wq = nc.dram_tensor("Wq", [DIM, DIM], BF16, kind="ExternalInput")
    bq = nc.dram_tensor("bq", [DIM], F32, kind="ExternalInput")
    wk = nc.dram_tensor("Wk", [DIM, DIM], BF16, kind="ExternalInput")
    bk = nc.dram_tensor("bk", [DIM], F32, kind="ExternalInput")
    wv = nc.dram_tensor("Wv", [DIM, DIM], BF16, kind="ExternalInput")
    bv = nc.dram_tensor("bv", [DIM], F32, kind="ExternalInput")
    wo = nc.dram_tensor("Wo", [DIM, DIM], BF16, kind="ExternalInput")
    bo = nc.dram_tensor("bo", [DIM], F32, kind="ExternalInput")
    out = nc.dram_tensor("out", [BL, N, DIM], F32, kind="ExternalOutput")

    wq_r = wq.rearrange("(ko ki) m -> ki ko m", ki=P)
    wk_r = wk.rearrange("(ko ki) m -> ki ko m", ki=P)
    wv_r = wv.rearrange("(ko ki) m -> ki ko m", ki=P)
    wo_r = wo.rearrange("(ko ki) m -> ki ko m", ki=P)
    bq_r = bq.rearrange("(ko ki) -> ki ko", ki=P)
    bk_r = bk.rearrange("(ko ki) -> ki ko", ki=P)

    XB = 3  # rotation depth for the small per-pair tiles

    with tile.TileContext(nc) as tc:
        with (
            tc.tile_pool(name="const", bufs=1) as cpool,
            tc.tile_pool(name="work", bufs=1) as pool,
            tc.tile_pool(name="dram", bufs=1, space="DRAM") as dpool,
            tc.tile_pool(name="ps", bufs=1, space="PSUM") as ps,
        ):
            # resident weights; Wv on the scalar HWDGE queue (needed first),
            # the rest on gpsimd so they don't delay it
            wv_sb = cpool.tile([P, KT, DIM], BF16)
            wq_sb = cpool.tile([P, KT, DIM], BF16)
            wk_sb = cpool.tile([P, KT, DIM], BF16)
            wo_sb = cpool.tile([P, KT, DIM], BF16)
            for k in range(KT):
                nc.scalar.dma_start(wv_sb[:, k], wv_r[:, k])
            for k in range(KT):
                nc.gpsimd.dma_start(wq_sb[:, k], wq_r[:, k])
                nc.gpsimd.dma_start(wk_sb[:, k], wk_r[:, k])
            for k in range(KT):
                nc.gpsimd.dma_start(wo_sb[:, k], wo_r[:, k])

            bv_b = cpool.tile([P, DIM], F32)
            bq_sb = cpool.tile([P, KT], F32)
            bk_sb = cpool.tile([P, KT], F32)
            bo_b = cpool.tile([P, DIM], F32)
            nc.scalar.dma_start(bv_b[:], bv[None, :].to_broadcast((P, DIM)))
            nc.scalar.dma_start(bq_sb[:], bq_r)
            nc.scalar.dma_start(bk_sb[:], bk_r)
            nc.scalar.dma_start(bo_b[:], bo[None, :].to_broadcast((P, DIM)))

            # per-batch resident tiles
            xt = [cpool.tile([P, KT, N], BF16, name=f"xt{b}") for b in range(BL)]
            ot = [cpool.tile([P, KT, N], BF16, name=f"ot{b}") for b in range(BL)]

            # V_ext: [tok_inner, tok_outer, pair blocks of PAIRW cols]
            # cols p*PAIRW + [0:64] = V head 2p, +64 = ones, +[96:160] = V 2p+1
            # pad cols stay uninitialized: they only produce garbage psum rows
            # that are never read. Ones col via DVE cast-copy.
            v_ext = [
                cpool.tile([P, TT, NPAIR * PAIRW], BF16, name=f"vext{b}")
                for b in range(BL)
            ]
            ones_src = cpool.tile([P, TT * NPAIR], F32)
            nc.vector.memset(ones_src[:], 1.0)
            for b in range(BL):
                ones_cols = v_ext[b][:].rearrange(
                    "p t (np w) -> p t np w", w=PAIRW
                )[:, :, :, 64:65]
                nc.vector.tensor_copy(
                    ones_cols,
                    ones_src[:].rearrange("p (t np) -> p t np", np=NPAIR)[
                        :, :, :, None
                    ],
                )

            # HAM pre-warm: the clock gate needs ~3.4us of sustained PE
            # activity to lift the PE 1.2->2.4 GHz, and real matmuls only
            # start at ~10us (DMA-gated). Burn the dead window with dummy
            # matmuls on already-resident data (output never read) so the
            # projection stream starts at full clock.
            warm_ps = ps.tile([P, QC], F32, tag="oa", bufs=1, name="warm")
            for w in range(48):
                nc.tensor.matmul(
                    warm_ps[0:48, 0:48],
                    ones_src[:, 0:48],
                    ones_src[:, 0:48],
                    start=(w == 0),
                    stop=(w == 47),
                )

            # ---- emission helpers --------------------------------------

            def emit_xt(b, to):
                # XBAR DMA transpose of one token tile: [128 tok, 768] ->
                # xt[b][:, :, to] (feat-major)
                nc.sync.dma_start_transpose(
                    xt[b][:, :, to * P : (to + 1) * P],
                    x[b, to * P : (to + 1) * P, :],
                )

            # psum tag rotation: during startup (V of batch 0) and tail (Y of
            # batch 1) the attention banks (st/oa/ob) are idle — rotating the
            # projection psum tiles across them removes the 2-bank
            # serialization against the DVE drain of the previous tile.
            PTAGS = [
                (("mm", 2, "vps0"), ("mm", 2, "vps1")),
                (("st", 2, "vps0s"), ("st", 2, "vps1s")),
                (("oa", 1, "vps0o"), ("ob", 1, "vps1o")),
            ]

            def emit_v(b, to, rot=0):
                # V natural for one token tile + scatter into v_ext[b]
                tags = PTAGS[to % 3] if rot else PTAGS[0]
                vpss = {
                    ch: ps.tile([P, QC], F32, tag=tags[ch][0], bufs=tags[ch][1],
                                name=tags[ch][2])
                    for ch in (0, 1)
                }
                for k in range(KT):
                    for ch, cw in ((0, 512), (1, 256)):
                        nc.tensor.matmul(
                            vpss[ch][:, :cw],
                            xt[b][:, k, to * P : (to + 1) * P],
                            wv_sb[:, k, ch * 512 : ch * 512 + cw],
                            start=(k == 0),
                            stop=(k == KT - 1),
                        )
                for ch, cw in ((0, 512), (1, 256)):
                    vps = vpss[ch]
                    npr = cw // (2 * HD)  # pairs in this chunk (4 then 2)
                    pr0 = ch * 4          # first pair in this chunk
                    for par in (0, 1):    # even/odd head of each pair
                        src = vps[:, :cw].rearrange(
                            "p (np two w) -> p np two w", two=2, w=HD
                        )[:, :, par, :]
                        bsrc = bv_b[:, ch * 512 : ch * 512 + cw].rearrange(
                            "p (np two w) -> p np two w", two=2, w=HD
                        )[:, :, par, :]
                        off = 96 if par else 0
                        dst = v_ext[b][:, to, :].rearrange(
                            "p (np w) -> p np w", w=PAIRW
                        )[:, pr0 : pr0 + npr, off : off + HD]
                        nc.vector.scalar_tensor_tensor(
                            out=dst,
                            in0=src,
                            scalar=1.0,
                            in1=bsrc,
                            op0=mybir.AluOpType.mult,
                            op1=mybir.AluOpType.add,
                        )

            def emit_qk(b, po):
                qt_t = pool.tile([P, N], BF16, tag="qt", bufs=XB)
                kt_t = pool.tile([P, N], BF16, tag="kt", bufs=XB)
                for dst_t, w_t, bias in (
                    (qt_t, wq_sb, bq_sb),
                    (kt_t, wk_sb, bk_sb),
                ):
                    # qs outer / k inner: each query-chunk holds ONE psum
                    # bank for its whole accumulation, so the two chunks
                    # (and other hosted projections) pipeline through the 2
                    # "mm" slots independently instead of locking both
                    for qs in range(N // QC):
                        pps = ps.tile([P, QC], F32, tag="mm", bufs=2,
                                      name=f"pps{qs}")
                        for k in range(KT):
                            nc.tensor.matmul(
                                pps[:],
                                w_t[:, k, po * P : (po + 1) * P],
                                xt[b][:, k, qs * QC : (qs + 1) * QC],
                                start=(k == 0),
                                stop=(k == KT - 1),
                            )
                        nc.vector.tensor_scalar_add(
                            dst_t[:, qs * QC : (qs + 1) * QC],
                            pps[:],
                            bias[:, po : po + 1],
                        )
                return qt_t, kt_t

            # Softmax epilogue, entirely OFF the DVE queue: the
            # reciprocal (Newton iteration, bit-trick seed) and the normalize
            # multiplies run on the otherwise-idle GPSIMD engine.
            # Rationale: both DVE and ACT queues are in-order and busy; a
            # latency-bound op (waiting on the denominator DMA roundtrip) at
            # the head of either queue convoys the psum-drain copies behind
            # it, which hold the PV psum banks, which starves the PE (a
            # measured 10-23us full-pipeline stall per batch). GPSIMD is ~3%
            # busy, SBUF-only, and nothing queues behind it, so the roundtrip
            # latency is absorbed there.
            I32 = mybir.dt.int32
            RMAGIC = 0x7EF311C3  # seed: bitcast(RMAGIC - bitcast(x)) ~ 1/x

            def finish_epi(e):
                den = e["den_sq"]  # [P, 8] f32, den per (head, query)
                rc0 = pool.tile([P, 8], F32, tag="rc0", bufs=2)
                rc1 = pool.tile([P, 8], F32, tag="rc1", bufs=2)
                rc2 = pool.tile([P, 8], F32, tag="rc2", bufs=2)
                t0 = pool.tile([P, 8], F32, tag="t0", bufs=2)
                t1 = pool.tile([P, 8], F32, tag="t1", bufs=2)
                nc.gpsimd.tensor_scalar(
                    out=rc0[:].bitcast(I32), in0=den[:].bitcast(I32),
                    scalar1=-1, scalar2=RMAGIC,
                    op0=mybir.AluOpType.mult, op1=mybir.AluOpType.add,
                )
                # two Newton steps: r <- r * (2 - x*r)
                nc.gpsimd.tensor_mul(t0[:], den[:], rc0[:])
                nc.gpsimd.tensor_scalar(
                    out=t0[:], in0=t0[:], scalar1=-1.0, scalar2=2.0,
                    op0=mybir.AluOpType.mult, op1=mybir.AluOpType.add,
                )
                nc.gpsimd.tensor_mul(rc1[:], rc0[:], t0[:])
                nc.gpsimd.tensor_mul(t1[:], den[:], rc1[:])
                nc.gpsimd.tensor_scalar(
                    out=t1[:], in0=t1[:], scalar1=-1.0, scalar2=2.0,
                    op0=mybir.AluOpType.mult, op1=mybir.AluOpType.add,
                )
                nc.gpsimd.tensor_mul(rc2[:], rc1[:], t1[:])
                drin = dpool.tile([2, QC], F32, tag="drin", bufs=2)
                nc.sync.dma_start(
                    drin[:].rearrange("a c -> (a c)").rearrange(
                        "(p f) -> p f", p=P
                    ),
                    rc2[:],
                )
                rb = pool.tile([P, QC], F32, tag="rb", bufs=XB)
                nc.sync.dma_start(
                    rb[0:64, :], drin[0:1, :].to_broadcast((64, QC))
                )
                nc.sync.dma_start(
                    rb[64:128, :], drin[1:2, :].to_broadcast((64, QC))
                )
                b, po, qsl = e["b"], e["po"], e["qsl"]
                nc.gpsimd.tensor_mul(
                    ot[b][0:64, po, qsl], e["osb_a"][0:64, :], rb[0:64, :]
                )
                nc.gpsimd.tensor_mul(
                    ot[b][64:128, po, qsl], e["osb_b"][64:128, :], rb[64:128, :]
                )

            def emit_attn(b, po, qt_t, kt_t):
                pb = po * PAIRW
                for qc in range(N // QC):
                    qsl = slice(qc * QC, (qc + 1) * QC)
                    oa = ps.tile([P, QC], F32, tag="oa", bufs=1, name="oa")
                    ob = ps.tile([P, QC], F32, tag="ob", bufs=1, name="ob")
                    for g in range(TT // 2):
                        st0 = ps.tile([P, 2 * QC], F32, tag="st", bufs=2, name="st0")
                        st1 = ps.tile([P, 2 * QC], F32, tag="st", bufs=2, name="st1")
                        for j in range(2):
                            kb = 2 * g + j
                            ksl = slice(kb * P, (kb + 1) * P)
                            nc.tensor.matmul(
                                st0[:, j * QC : (j + 1) * QC],
                                kt_t[0:64, ksl],
                                qt_t[0:64, qsl],
                                start=True,
                                stop=True,
                            )
                            nc.tensor.matmul(
                                st1[:, j * QC : (j + 1) * QC],
                                kt_t[64:128, ksl],
                                qt_t[64:128, qsl],
                                start=True,
                                stop=True,
                            )
                        pt0 = pool.tile([P, 2 * QC], BF16, tag="pt0", bufs=XB)
                        pt1 = pool.tile([P, 2 * QC], BF16, tag="pt1", bufs=XB)
                        nc.scalar.activation(
                            pt0[:], st0[:], mybir.ActivationFunctionType.Exp,
                            scale=SCALE,
                        )
                        nc.scalar.activation(
                            pt1[:], st1[:], mybir.ActivationFunctionType.Exp,
                            scale=SCALE,
                        )
                        for j in range(2):
                            kb = 2 * g + j
                            first = g == 0 and j == 0
                            last = g == TT // 2 - 1 and j == 1
                            nc.tensor.matmul(
                                oa[:, :],
                                v_ext[b][:, kb, pb : pb + 128],
                                pt0[:, j * QC : (j + 1) * QC],
                                start=first,
                                stop=last,
                            )
                            nc.tensor.matmul(
                                ob[:, :],
                                v_ext[b][:, kb, pb + 32 : pb + 160],
                                pt1[:, j * QC : (j + 1) * QC],
                                start=first,
                                stop=last,
                            )
                    # epilogue part 1 (inline): drain psum (frees oa/ob
                    # banks) and kick off the denominator DMA roundtrip
                    osb_a = pool.tile([P, QC], F32, tag="osb_a", bufs=XB)
                    osb_b = pool.tile([P, QC], F32, tag="osb_b", bufs=XB)
                    nc.vector.tensor_copy(osb_a[0:65, :], oa[0:65, :])
                    nc.vector.tensor_copy(osb_b[64:128, :], ob[64:128, :])
                    nc.vector.tensor_copy(osb_b[32:33, :], ob[32:33, :])
                    # denominators -> DRAM, then re-read as [128, 8] (all-
                    # partition layout for the reciprocal; DRAM linearizes
                    # the cross-partition reshape)
                    dden = dpool.tile([2, QC], F32, tag="dden", bufs=XB)
                    nc.sync.dma_start(dden[0:1, :], osb_a[64:65, :])
                    nc.sync.dma_start(dden[1:2, :], osb_b[32:33, :])
                    den_sq = pool.tile([P, 8], F32, tag="den_sq", bufs=XB)
                    nc.sync.dma_start(
                        den_sq[:],
                        dden[:].rearrange("a c -> (a c)").rearrange(
                            "(p f) -> p f", p=P
                        ),
                    )
                    finish_epi(dict(
                        b=b, po=po, qsl=qsl, osb_a=osb_a, osb_b=osb_b,
                        den_sq=den_sq,
                    ))

            def emit_y(b, to, rot=0):
                ystage = pool.tile([P, DIM], F32, tag="ystage", bufs=XB)
                tags = PTAGS[to % 3] if rot else PTAGS[0]
                ypss = {
                    ch: ps.tile([P, QC], F32, tag=tags[ch][0], bufs=tags[ch][1],
                                name=tags[ch][2])
                    for ch in (0, 1)
                }
                for k in range(KT):
                    for ch, cw in ((0, 512), (1, 256)):
                        nc.tensor.matmul(
                            ypss[ch][:, :cw],
                            ot[b][:, k, to * P : (to + 1) * P],
                            wo_sb[:, k, ch * 512 : ch * 512 + cw],
                            start=(k == 0),
                            stop=(k == KT - 1),
                        )
                for ch, cw in ((0, 512), (1, 256)):
                    nc.vector.scalar_tensor_tensor(
                        out=ystage[:, ch * 512 : ch * 512 + cw],
                        in0=ypss[ch][:, :cw],
                        scalar=1.0,
                        in1=bo_b[:, ch * 512 : ch * 512 + cw],
                        op0=mybir.AluOpType.mult,
                        op1=mybir.AluOpType.add,
                    )
                nc.scalar.dma_start(out[b, to * P : (to + 1) * P, :], ystage[:])

            # ---- schedule (software-pipelined across the 2 batches) ----

            # how many of the other batch's V / Y token-tiles to host after
            # each attention pair (8 tiles spread over 6 pairs)
            HOSTED = [2, 2, 1, 1, 1, 1]
            assert sum(HOSTED) == TT

            for to in range(TT):
                emit_xt(0, to)
            for to in range(TT):
                emit_v(0, to, rot=1)
            # batch-1 transposes early, clumped so the XBAR stays in
            # transpose mode in one stretch on the sync queue
            for to in range(TT):
                emit_xt(1, to)

            qk = emit_qk(0, 0)
            nxt = 0
            for po in range(NPAIR):
                emit_attn(0, po, *qk)
                if po + 1 < NPAIR:
                    qk = emit_qk(0, po + 1)
                for _ in range(HOSTED[po]):
                    emit_v(1, nxt)
                    nxt += 1

            qk = emit_qk(1, 0)
            nxt = 0
            for po in range(NPAIR):
                emit_attn(1, po, *qk)
                if po + 1 < NPAIR:
                    qk = emit_qk(1, po + 1)
                for _ in range(HOSTED[po]):
                    emit_y(0, nxt)
                    nxt += 1

            for to in range(TT):
                emit_y(1, to, rot=1)

    nc.finalize()
    return nc


def _run(inputs: dict, mm_dtype=None, attn_bf16=True, trace: bool = False, dbg: bool = False):
    if "bf16" not in _cache:
        _cache["bf16"] = build()
    nc = _cache["bf16"]

    np_bf16 = mybir.dt.np(BF16)
    x = np.ascontiguousarray(inputs["inputs"]).astype(np_bf16)
    shared = {}
    for k in ("Wq", "Wk", "Wv", "Wo"):
        shared[k] = np.ascontiguousarray(inputs[k]).astype(np_bf16)
    for k in ("bq", "bk", "bv", "bo"):
        shared[k] = np.ascontiguousarray(inputs[k], dtype=np.float32)
    in_maps = [
        {"inputs": x[c * BL : (c + 1) * BL], **shared} for c in range(NCORES)
    ]
    res = run_bass_kernel_spmd(nc, in_maps, list(range(NCORES)), trace=trace)
    full = np.concatenate([res.results[c]["out"] for c in range(NCORES)], axis=0)
    return full, res


def kernel(**inputs) -> np.ndarray:
    out, _ = _run(inputs)
    return out
